# revision 3
# baseline (speedup 1.0000x reference)
"""Trainium2 Bass kernel: 2-layer GCN encoder (BN -> GCNConv -> BN -> ReLU
-> GCNConv -> BN -> ReLU -> linear mu / log_std heads) on 8 NeuronCores.

Self-contained: kernel(**inputs) takes full inputs, shards internally
(1D node sharding, edges partitioned by destination), runs one SPMD Bass
program on cores 0-7, returns full (mu, log_std).
"""
import sys

if "/opt/trn_rl_repo" not in sys.path:
    sys.path.insert(0, "/opt/trn_rl_repo")

import numpy as np

N = 50000
NC = 8
P = 128
NPC = 6250
WINS = 49
SLOTS = WINS * P        # 6272
NTBL = NC * SLOTS       # 50176
JFULL = NTBL // P       # 392
EPS = 1e-5
A_HI = 32768
B_LO = NTBL - 32768     # 17408
G = 7                   # windows per gather batch
NB = WINS // G          # 7


def q_of(i):
    return (i // NPC) * SLOTS + (i % NPC)


def preprocess(edge_index, edge_weight):
    """Build per-core chunk structure. Returns (percore list of dicts, CA, CB)."""
    src = np.asarray(edge_index[0], np.int64)
    dst = np.asarray(edge_index[1], np.int64)
    w = np.asarray(edge_weight, np.float32)
    loop = np.arange(N, dtype=np.int64)
    src = np.concatenate([src, loop])
    dst = np.concatenate([dst, loop])
    w = np.concatenate([w, np.ones(N, np.float32)])
    qsrc = q_of(src)

    core = dst // NPC
    wloc = (dst % NPC) // P
    dloc = (dst % NPC) % P

    per = []
    for c in range(NC):
        m = core == c
        qs_c, dl_c, w_c, wl_c = qsrc[m], dloc[m], w[m], wloc[m]
        o = np.lexsort((qs_c, wl_c))
        qs_c, dl_c, w_c, wl_c = qs_c[o], dl_c[o], w_c[o], wl_c[o]
        wins = []
        start = np.searchsorted(wl_c, np.arange(WINS))
        end = np.searchsorted(wl_c, np.arange(WINS) + 1)
        for wi in range(WINS):
            qs = qs_c[start[wi]:end[wi]]
            dl = dl_c[start[wi]:end[wi]]
            ww = w_c[start[wi]:end[wi]]
            chunks = []
            i, n = 0, len(qs)
            while i < n:
                j = min(i + P, n)
                cqs = qs[i:j]
                if cqs[-1] < A_HI:
                    tag = "A"
                elif cqs[0] >= B_LO:
                    tag = "B"
                else:
                    j = i + int(np.searchsorted(cqs, A_HI))
                    assert j > i
                    cqs = qs[i:j]
                    tag = "A"
                chunks.append((tag, cqs, dl[i:j], ww[i:j]))
                i = j
            wins.append(chunks)
        per.append(wins)

    CA = max(sum(1 for t, *_ in win if t == "A") for wins in per for win in wins)
    CB = max(sum(1 for t, *_ in win if t == "B") for wins in per for win in wins)

    data = []
    for c in range(NC):
        idxA = np.zeros((WINS, CA, P), np.int32)
        dlA = np.zeros((WINS, CA, P), np.float32)
        wA = np.zeros((WINS, CA, P), np.float32)
        idxB = np.zeros((WINS, CB, P), np.int32)
        dlB = np.zeros((WINS, CB, P), np.float32)
        wB = np.zeros((WINS, CB, P), np.float32)
        for wi in range(WINS):
            ka = kb = 0
            for tag, cqs, cdl, cw in per[c][wi]:
                n = len(cqs)
                if tag == "A":
                    idxA[wi, ka, :n] = cqs
                    dlA[wi, ka, :n] = cdl
                    wA[wi, ka, :n] = cw
                    ka += 1
                else:
                    idxB[wi, kb, :n] = cqs - B_LO
                    dlB[wi, kb, :n] = cdl
                    wB[wi, kb, :n] = cw
                    kb += 1
        data.append(dict(idxA=idxA, dlA=dlA, wA=wA, idxB=idxB, dlB=dlB, wB=wB))
    return data, CA, CB


def wrap_idx16(idx_flat):
    """Edge-slot-ordered flat indices -> dma_gather int16 layout [128, n/16]."""
    n = len(idx_flat)
    assert n % 16 == 0
    base = idx_flat.reshape(n // 16, 16).T.astype(np.int16)   # [16, n/16]
    return np.tile(base, (8, 1))                               # [128, n/16]


def build_in_maps(inputs, data, CA, CB):
    h = np.asarray(inputs["h"], np.float32)
    hfull = np.zeros((NTBL, 5), np.float32)
    for c in range(NC):
        hfull[c * SLOTS : c * SLOTS + NPC] = h[c * NPC : (c + 1) * NPC]

    vecs = np.zeros((1, 8 * P), np.float32)
    vecs[0, 0*P:1*P] = np.asarray(inputs["g1"], np.float32)
    vecs[0, 1*P:2*P] = np.asarray(inputs["be1"], np.float32)
    vecs[0, 2*P:3*P] = np.asarray(inputs["g2"], np.float32)
    vecs[0, 3*P:4*P] = np.asarray(inputs["be2"], np.float32)
    vecs[0, 4*P:5*P] = np.asarray(inputs["bmu"], np.float32)
    vecs[0, 5*P:6*P] = np.asarray(inputs["bls"], np.float32)
    vecs[0, 6*P:6*P+5] = np.asarray(inputs["g0"], np.float32)
    vecs[0, 7*P:7*P+5] = np.asarray(inputs["be0"], np.float32)

    W1 = np.asarray(inputs["W1"], np.float32)              # [5,128]
    W2 = np.asarray(inputs["W2"], np.float32)              # [128,128]
    W2bf = W2.astype(np.dtype("bfloat16") if False else np.float32)
    Wmu = np.asarray(inputs["Wmu"], np.float32)
    Wls = np.asarray(inputs["Wls"], np.float32)

    in_maps = []
    for c in range(NC):
        d = data[c]
        # meta layout: [dlA | wA | dlB | wB] columns, window-major chunk order.
        # transposed to [128 rows(edges of chunk), cols(chunks)]
        def t(x, CC):
            return x.reshape(WINS * CC, P).T.copy()   # [128, WINS*CC]
        meta = np.concatenate(
            [t(d["dlA"], CA), t(d["wA"], CA), t(d["dlB"], CB), t(d["wB"], CB)],
            axis=1,
        ).astype(np.float32)
        idxA16 = wrap_idx16(d["idxA"].ravel())
        idxB16 = wrap_idx16(d["idxB"].ravel())
        in_maps.append({
            "hfull": hfull,
            "meta": meta,
            "idxA": idxA16,
            "idxB": idxB16,
            "W1": W1,
            "W2": W2,
            "Wmu": Wmu,
            "Wls": Wls,
            "vecs": vecs,
        })
    return in_maps


def build_kernel(CA, CB, debug=False, stage=5):
    import concourse.bass as bass
    import concourse.bacc as bacc
    import concourse.tile as tile
    from concourse import mybir
    from concourse.masks import make_identity

    f32 = mybir.dt.float32
    bf16 = mybir.dt.bfloat16
    i16 = mybir.dt.int16
    AOT = mybir.AluOpType

    nc = bacc.Bacc("TRN2", num_devices=NC)

    # ---- I/O ----
    hfull_d = nc.dram_tensor("hfull", [NTBL, 5], f32, kind="ExternalInput")
    meta_d = nc.dram_tensor("meta", [P, 2 * WINS * (CA + CB)], f32, kind="ExternalInput")
    idxA_d = nc.dram_tensor("idxA", [P, WINS * CA * 8], i16, kind="ExternalInput")
    idxB_d = nc.dram_tensor("idxB", [P, WINS * CB * 8], i16, kind="ExternalInput")
    W1_d = nc.dram_tensor("W1", [5, P], f32, kind="ExternalInput")
    W2_d = nc.dram_tensor("W2", [P, P], f32, kind="ExternalInput")
    Wmu_d = nc.dram_tensor("Wmu", [P, P], f32, kind="ExternalInput")
    Wls_d = nc.dram_tensor("Wls", [P, P], f32, kind="ExternalInput")
    vecs_d = nc.dram_tensor("vecs", [1, 8 * P], f32, kind="ExternalInput")
    mu_d = nc.dram_tensor("mu_out", [SLOTS, P], f32, kind="ExternalOutput")
    ls_d = nc.dram_tensor("ls_out", [SLOTS, P], f32, kind="ExternalOutput")
    if debug:
        dbg_dis_d = nc.dram_tensor("dbg_dis", [P, WINS], f32, kind="ExternalOutput")
        dbg_out1_d = nc.dram_tensor("dbg_out1", [SLOTS, P], f32, kind="ExternalOutput")
        dbg_x1_d = nc.dram_tensor("dbg_x1", [SLOTS, P], f32, kind="ExternalOutput")

    # meta column offsets
    oDlA, oWA = 0, WINS * CA
    oDlB, oWB = 2 * WINS * CA, 2 * WINS * CA + WINS * CB

    class StopStage(Exception):
        pass

    with tile.TileContext(nc) as tc:
        with (
            tc.tile_pool(name="const", bufs=1) as cp,
            tc.tile_pool(name="store", bufs=1) as st,
            tc.tile_pool(name="work", bufs=3) as wk,
            tc.tile_pool(name="spool", bufs=3) as sp,
            tc.tile_pool(name="sbig", bufs=2) as sb2,
            tc.tile_pool(name="psum", bufs=2, space="PSUM") as ps,
            tc.tile_pool(name="dram", bufs=1, space="DRAM") as dr,
        ):
          try:
            # ---------- constants ----------
            iota_f = cp.tile([P, P], f32)
            nc.gpsimd.iota(iota_f[:], pattern=[[1, P]], base=0,
                           channel_multiplier=0, allow_small_or_imprecise_dtypes=True)
            CMX = max(CA, CB)
            iota_big = cp.tile([P, CMX, P], f32)
            nc.gpsimd.iota(iota_big[:], pattern=[[0, CMX], [1, P]], base=0,
                           channel_multiplier=0, allow_small_or_imprecise_dtypes=True)

            def s_batch(oDl, oW, wi, CC, tag):
                """Batched one-hot: S[:, k, :] = (iota==dl_k) * w_k for CC chunks."""
                dlb = meta_t[:, oDl + wi * CC : oDl + (wi + 1) * CC].rearrange(
                    "p (k o) -> p k o", o=1).broadcast_to([P, CC, P])
                wb = meta_t[:, oW + wi * CC : oW + (wi + 1) * CC].rearrange(
                    "p (k o) -> p k o", o=1).broadcast_to([P, CC, P])
                tmp = sp.tile([P, CMX, P], f32, tag="stmp")
                nc.vector.tensor_tensor(tmp[:, :CC, :], iota_big[:, :CC, :], dlb,
                                        op=AOT.is_equal)
                sb_t = sp.tile([P, CC, P], bf16, tag=tag)
                nc.vector.tensor_tensor(sb_t[:], tmp[:, :CC, :], wb, op=AOT.mult)
                return sb_t
            ident = cp.tile([P, P], f32)
            make_identity(nc, ident[:])
            ones_bf = cp.tile([P, 1], bf16)
            nc.gpsimd.memset(ones_bf[:], 1.0)
            ones_f = cp.tile([P, 1], f32)
            nc.gpsimd.memset(ones_f[:], 1.0)
            one_row = cp.tile([1, P], f32)
            nc.gpsimd.memset(one_row[:], 1.0)

            meta_t = cp.tile([P, 2 * WINS * (CA + CB)], f32)
            nc.sync.dma_start(meta_t[:], meta_d[:])
            idxA_t = cp.tile([P, WINS * CA * 8], i16)
            nc.sync.dma_start(idxA_t[:], idxA_d[:])
            idxB_t = cp.tile([P, WINS * CB * 8], i16)
            nc.sync.dma_start(idxB_t[:], idxB_d[:])
            W1_t = cp.tile([5, P], f32)
            nc.sync.dma_start(W1_t[:], W1_d[:])
            W2_t = cp.tile([P, P], f32)
            nc.sync.dma_start(W2_t[:], W2_d[:])
            Wmu_t = cp.tile([P, P], f32)
            nc.sync.dma_start(Wmu_t[:], Wmu_d[:])
            Wls_t = cp.tile([P, P], f32)
            nc.sync.dma_start(Wls_t[:], Wls_d[:])
            vecs_t = cp.tile([1, 8 * P], f32)
            nc.sync.dma_start(vecs_t[:], vecs_d[:])

            # absorb DMA waits on DVE (TensorScalarPtr allows only 1 wait)
            touch = cp.tile([P, 1], f32)
            nc.vector.tensor_copy(touch[:], meta_t[:, 0:1])

            # ---------- BN0 stats from full h (replicated) ----------
            hfull_t = cp.tile([P, JFULL, 5], f32)
            nc.sync.dma_start(hfull_t[:], hfull_d[:].rearrange("(j p) d -> p j d", p=P))
            hsq = wk.tile([P, JFULL * 5], f32, tag="hsq")
            nc.scalar.square(hsq[:], hfull_t[:].rearrange("p j d -> p (j d)"))
            part_s = wk.tile([P, 5], f32, tag="part")
            nc.vector.tensor_reduce(
                part_s[:], hfull_t[:].rearrange("p j d -> p d j"),
                axis=mybir.AxisListType.X, op=AOT.add)
            part_q = wk.tile([P, 5], f32, tag="part")
            nc.vector.tensor_reduce(
                part_q[:], hsq[:].rearrange("p (j d) -> p d j", d=5),
                axis=mybir.AxisListType.X, op=AOT.add)
            s0_ps = ps.tile([1, 5], f32, space="PSUM", tag="sps")
            nc.tensor.matmul(s0_ps[:], lhsT=ones_f[:], rhs=part_s[:], start=True, stop=True)
            q0_ps = ps.tile([1, 5], f32, space="PSUM", tag="sps")
            nc.tensor.matmul(q0_ps[:], lhsT=ones_f[:], rhs=part_q[:], start=True, stop=True)

            # a0 = g0 * rsqrt(v0+eps), c0 = be0 - m0*a0   on [1,5]
            m0 = cp.tile([1, 5], f32)
            nc.vector.tensor_scalar(m0[:], s0_ps[:], 1.0 / N, None, op0=AOT.mult)
            v0 = cp.tile([1, 5], f32)
            nc.vector.tensor_scalar(v0[:], q0_ps[:], 1.0 / N, None, op0=AOT.mult)
            m0sq = wk.tile([1, 5], f32, tag="t5")
            nc.vector.tensor_tensor(m0sq[:], m0[:], m0[:], op=AOT.mult)
            nc.vector.tensor_tensor(v0[:], v0[:], m0sq[:], op=AOT.subtract)
            nc.vector.tensor_scalar(v0[:], v0[:], EPS, None, op0=AOT.add)
            rc0 = wk.tile([1, 5], f32, tag="t5")
            nc.vector.reciprocal(rc0[:], v0[:])
            rs0 = wk.tile([1, 5], f32, tag="t5")
            nc.scalar.sqrt(rs0[:], rc0[:])
            a0 = cp.tile([1, 5], f32)
            nc.vector.tensor_tensor(a0[:], rs0[:], vecs_t[0:1, 6*P:6*P+5], op=AOT.mult)
            c0 = cp.tile([1, 5], f32)
            nc.vector.tensor_tensor(c0[:], m0[:], a0[:], op=AOT.mult)
            nc.vector.tensor_tensor(c0[:], vecs_t[0:1, 7*P:7*P+5], c0[:], op=AOT.subtract)

            # broadcast a0,c0 to [128, 5] via outer product with ones
            a0f_ps = ps.tile([P, 5], f32, space="PSUM", tag="mmout")
            nc.tensor.matmul(a0f_ps[:], lhsT=one_row[:], rhs=a0[:], start=True, stop=True)
            a0_full = cp.tile([P, 5], f32)
            nc.vector.tensor_copy(a0_full[:], a0f_ps[:])
            c0f_ps = ps.tile([P, 5], f32, space="PSUM", tag="mmout")
            nc.tensor.matmul(c0f_ps[:], lhsT=one_row[:], rhs=c0[:], start=True, stop=True)
            c0_full = cp.tile([P, 5], f32)
            nc.vector.tensor_copy(c0_full[:], c0f_ps[:])

            # ---------- own-slice x0 ----------
            # hfull rows [rank*SLOTS, (rank+1)*SLOTS) -- but rank differs per core!
            # We avoid rank-dependence: each core's OWN slice in hfull is
            # provided via a separate per-core input tensor "hown".
            # (declared below, appended to I/O)

            # ---------- deg pass ----------
            deg_t = cp.tile([P, WINS], f32)
            for wi in range(WINS):
                dps = ps.tile([P, 1], f32, space="PSUM", tag="sps")
                sA = s_batch(oDlA, oWA, wi, CA, "sa")
                sB = s_batch(oDlB, oWB, wi, CB, "sb")
                nci = 0
                for k in range(CA):
                    nc.tensor.matmul(dps[:], lhsT=sA[:, k, :], rhs=ones_bf[:],
                                     start=(nci == 0), stop=False)
                    nci += 1
                for k in range(CB):
                    nci += 1
                    nc.tensor.matmul(dps[:], lhsT=sB[:, k, :], rhs=ones_bf[:],
                                     start=False, stop=(nci == CA + CB))
                nc.vector.tensor_copy(deg_t[:, wi : wi + 1], dps[:])

            # dis = (deg>0) * sqrt(1/max(deg,1e-12))   [128, WINS]
            degm = wk.tile([P, WINS], f32, tag="degm")
            nc.vector.tensor_scalar(degm[:], deg_t[:], 1e-12, None, op0=AOT.max)
            rec = wk.tile([P, WINS], f32, tag="degm")
            nc.vector.reciprocal(rec[:], degm[:])
            dsq = wk.tile([P, WINS], f32, tag="degm")
            nc.scalar.sqrt(dsq[:], rec[:])
            mask = wk.tile([P, WINS], f32, tag="degm")
            nc.vector.tensor_scalar(mask[:], deg_t[:], 0.0, None, op0=AOT.is_gt)
            dis_t = cp.tile([P, WINS], f32)
            nc.vector.tensor_tensor(dis_t[:], dsq[:], mask[:], op=AOT.mult)
            if debug:
                nc.sync.dma_start(dbg_dis_d[:], dis_t[:])

            if stage < 2:
                raise StopStage
            # ---------- helper: transform window (transpose + matmul) ----------
            def transform(src_sb, rhs_list):
                """src_sb [128, K] fp32 -> PE transpose -> [K,128] -> matmuls.
                Returns list of PSUM tiles [128, 128]."""
                kdim = src_sb.shape[-1]
                tps = ps.tile([P, P], f32, space="PSUM", tag="tps")
                nc.tensor.transpose(tps[:kdim, :], src_sb, ident[:])
                tsb = wk.tile([P, P], f32, tag="tsb")
                nc.vector.tensor_copy(tsb[:kdim, :], tps[:kdim, :])
                outs = []
                for rhs in rhs_list:
                    mps = ps.tile([P, P], f32, space="PSUM", tag="mmout")
                    nc.tensor.matmul(mps[:], lhsT=tsb[:kdim, :], rhs=rhs,
                                     start=True, stop=True)
                    outs.append(mps)
                return outs

            # ---------- z table (layer-1, transform-first) ----------
            hown_d = nc.dram_tensor("hown", [SLOTS, 5], f32, kind="ExternalInput")
            hown_t = cp.tile([P, WINS, 5], f32)
            nc.sync.dma_start(hown_t[:], hown_d[:].rearrange("(w p) d -> p w d", p=P))
            x0_t = cp.tile([P, WINS, 5], f32)
            for dd in range(5):
                nc.vector.tensor_scalar(
                    x0_t[:, :, dd], hown_t[:, :, dd],
                    a0_full[:, dd : dd + 1], c0_full[:, dd : dd + 1],
                    op0=AOT.mult, op1=AOT.add)

            ag_in1 = dr.tile([SLOTS, P], bf16)
            tbl1 = dr.tile([NTBL, P], bf16, addr_space="Shared")
            for wi in range(WINS):
                (zps,) = transform(x0_t[:, wi, :], [W1_t[:]])
                zbf = wk.tile([P, P], bf16, tag="zbf")
                nc.vector.tensor_scalar(
                    zbf[:], zps[:], dis_t[:, wi : wi + 1], None, op0=AOT.mult)
                nc.sync.dma_start(ag_in1[wi * P : (wi + 1) * P, :], zbf[:])
            nc.gpsimd.collective_compute(
                "AllGather", AOT.bypass, replica_groups=[list(range(NC))],
                ins=[ag_in1[:]], outs=[tbl1[:]])

            if stage == 2.5:
                # probe: single gather from AG output, dump
                dbgA_d = nc.dram_tensor("dbgA", [P, G * CA, P], f32, kind="ExternalOutput")
                bufA = sb2.tile([P, G * CA, P], bf16, tag="bufA")
                nc.gpsimd.dma_gather(
                    bufA[:], tbl1[:], idxA_t[:, 0 : G * CA * 8],
                    G * CA * P, G * CA * P, P, single_packet=False)
                nc.gpsimd.dma_start(dbgA_d[:], bufA[:])
                dbgB_d = nc.dram_tensor("dbgB", [P, G * CB, P], f32, kind="ExternalOutput")
                bufB = sb2.tile([P, G * CB, P], bf16, tag="bufB")
                nc.gpsimd.dma_gather(
                    bufB[:], tbl1[B_LO:, :], idxB_t[:, 0 : G * CB * 8],
                    G * CB * P, G * CB * P, P, single_packet=False)
                nc.gpsimd.dma_start(dbgB_d[:], bufB[:])
            if stage < 3:
                raise StopStage
            # ---------- aggregation pass ----------
            out_store = st.tile([P, WINS, P], f32)

            def agg_pass(tbl):
                sum_acc = wk.tile([1, P], f32, tag="sacc")
                sq_acc = wk.tile([1, P], f32, tag="qacc")
                nc.gpsimd.memset(sum_acc[:], 0.0)
                nc.gpsimd.memset(sq_acc[:], 0.0)
                for b in range(NB):
                    w0 = b * G
                    bufA = sb2.tile([P, G * CA, P], bf16, tag="bufA")
                    nc.gpsimd.dma_gather(
                        bufA[:], tbl[:],
                        idxA_t[:, w0 * CA * 8 : (w0 + G) * CA * 8],
                        G * CA * P, G * CA * P, P, single_packet=False)
                    bufB = sb2.tile([P, G * CB, P], bf16, tag="bufB")
                    nc.gpsimd.dma_gather(
                        bufB[:], tbl[B_LO:, :],
                        idxB_t[:, w0 * CB * 8 : (w0 + G) * CB * 8],
                        G * CB * P, G * CB * P, P, single_packet=False)
                    for wi in range(w0, w0 + G):
                        agg = ps.tile([P, P], f32, space="PSUM", tag="agg")
                        sA = s_batch(oDlA, oWA, wi, CA, "sa")
                        sB = s_batch(oDlB, oWB, wi, CB, "sb")
                        nci = 0
                        for k in range(CA):
                            nc.tensor.matmul(
                                agg[:], lhsT=sA[:, k, :],
                                rhs=bufA[:, (wi - w0) * CA + k, :],
                                start=(nci == 0), stop=False)
                            nci += 1
                        for k in range(CB):
                            nci += 1
                            nc.tensor.matmul(
                                agg[:], lhsT=sB[:, k, :],
                                rhs=bufB[:, (wi - w0) * CB + k, :],
                                start=False, stop=(nci == CA + CB))
                        # out = dis * agg
                        outw = out_store[:, wi, :]
                        nc.vector.tensor_scalar(
                            outw, agg[:], dis_t[:, wi : wi + 1], None, op0=AOT.mult)
                        # stats
                        sq = wk.tile([P, P], f32, tag="sq")
                        nc.scalar.square(sq[:], outw)
                        sps = ps.tile([1, P], f32, space="PSUM", tag="sps")
                        nc.tensor.matmul(sps[:], lhsT=ones_f[:], rhs=outw,
                                         start=True, stop=True)
                        nc.vector.tensor_tensor(sum_acc[:], sum_acc[:], sps[:], op=AOT.add)
                        qps = ps.tile([1, P], f32, space="PSUM", tag="sps")
                        nc.tensor.matmul(qps[:], lhsT=ones_f[:], rhs=sq[:],
                                         start=True, stop=True)
                        nc.vector.tensor_tensor(sq_acc[:], sq_acc[:], qps[:], op=AOT.add)
                return sum_acc, sq_acc

            def bn_reduce(sum_acc, sq_acc, g_row, be_row, name):
                """AllReduce stats; returns (a_full, c_full) [128,128] bcast tiles."""
                bn_in = dr.tile([1, 2 * P], f32, name=f"bnin_{name}")
                bn_out = dr.tile([1, 2 * P], f32, addr_space="Shared", name=f"bnout_{name}")
                pack = wk.tile([1, 2 * P], f32, tag="bnpack")
                nc.vector.tensor_copy(pack[0:1, 0:P], sum_acc[:])
                nc.vector.tensor_copy(pack[0:1, P : 2 * P], sq_acc[:])
                nc.sync.dma_start(bn_in[:], pack[:])
                nc.gpsimd.collective_compute(
                    "AllReduce", AOT.add, replica_groups=[list(range(NC))],
                    ins=[bn_in[:]], outs=[bn_out[:]])
                bn_t = wk.tile([1, 2 * P], f32, tag="bnt")
                nc.sync.dma_start(bn_t[:], bn_out[:])
                mean = wk.tile([1, P], f32, tag="bn1")
                nc.vector.tensor_scalar(mean[:], bn_t[0:1, 0:P], 1.0 / N, None, op0=AOT.mult)
                var = wk.tile([1, P], f32, tag="bn2")
                nc.vector.tensor_scalar(var[:], bn_t[0:1, P : 2 * P], 1.0 / N, None, op0=AOT.mult)
                msq = wk.tile([1, P], f32, tag="bn3")
                nc.vector.tensor_tensor(msq[:], mean[:], mean[:], op=AOT.mult)
                nc.vector.tensor_tensor(var[:], var[:], msq[:], op=AOT.subtract)
                nc.vector.tensor_scalar(var[:], var[:], EPS, None, op0=AOT.add)
                rc = wk.tile([1, P], f32, tag="bn3")
                nc.vector.reciprocal(rc[:], var[:])
                rs = wk.tile([1, P], f32, tag="bn3")
                nc.scalar.sqrt(rs[:], rc[:])
                a_row = wk.tile([1, P], f32, tag="bn4")
                nc.vector.tensor_tensor(a_row[:], rs[:], g_row, op=AOT.mult)
                c_row = wk.tile([1, P], f32, tag="bn5")
                nc.vector.tensor_tensor(c_row[:], mean[:], a_row[:], op=AOT.mult)
                nc.vector.tensor_tensor(c_row[:], be_row, c_row[:], op=AOT.subtract)
                af_ps = ps.tile([P, P], f32, space="PSUM", tag="mmout")
                nc.tensor.matmul(af_ps[:], lhsT=one_row[:], rhs=a_row[:], start=True, stop=True)
                a_full = st.tile([P, P], f32, name=f"afull_{name}")
                nc.vector.tensor_copy(a_full[:], af_ps[:])
                cf_ps = ps.tile([P, P], f32, space="PSUM", tag="mmout")
                nc.tensor.matmul(cf_ps[:], lhsT=one_row[:], rhs=c_row[:], start=True, stop=True)
                c_full = st.tile([P, P], f32, name=f"cfull_{name}")
                nc.vector.tensor_copy(c_full[:], cf_ps[:])
                return a_full, c_full

            # ----- layer 1 -----
            sum1, sq1 = agg_pass(tbl1)
            a1f, c1f = bn_reduce(sum1, sq1, vecs_t[0:1, 0:P], vecs_t[0:1, P:2*P], "bn1")
            if debug:
                nc.sync.dma_start(
                    dbg_out1_d[:].rearrange("(w p) d -> p w d", p=P), out_store[:])

            if stage < 4:
                raise StopStage
            ag_in2 = dr.tile([SLOTS, P], bf16)
            tbl2 = dr.tile([NTBL, P], bf16, addr_space="Shared")
            for wi in range(WINS):
                x1w = wk.tile([P, P], f32, tag="x1w")
                nc.vector.tensor_tensor(x1w[:], out_store[:, wi, :], a1f[:], op=AOT.mult)
                nc.vector.tensor_tensor(x1w[:], x1w[:], c1f[:], op=AOT.add)
                nc.vector.tensor_scalar(x1w[:], x1w[:], 0.0, None, op0=AOT.max)
                if debug:
                    nc.sync.dma_start(dbg_x1_d[wi * P : (wi + 1) * P, :], x1w[:])
                (w2ps,) = transform(x1w[:], [W2_t[:]])
                tbf = wk.tile([P, P], bf16, tag="zbf")
                nc.vector.tensor_scalar(
                    tbf[:], w2ps[:], dis_t[:, wi : wi + 1], None, op0=AOT.mult)
                nc.sync.dma_start(ag_in2[wi * P : (wi + 1) * P, :], tbf[:])
            nc.gpsimd.collective_compute(
                "AllGather", AOT.bypass, replica_groups=[list(range(NC))],
                ins=[ag_in2[:]], outs=[tbl2[:]])

            # ----- layer 2 -----
            sum2, sq2 = agg_pass(tbl2)
            a2f, c2f = bn_reduce(sum2, sq2, vecs_t[0:1, 2*P:3*P], vecs_t[0:1, 3*P:4*P], "bn2")

            if stage < 5:
                raise StopStage
            # bias rows for heads -> bcast tiles
            bmu_ps = ps.tile([P, P], f32, space="PSUM", tag="mmout")
            nc.tensor.matmul(bmu_ps[:], lhsT=one_row[:], rhs=vecs_t[0:1, 4*P:5*P], start=True, stop=True)
            bmu_f = st.tile([P, P], f32)
            nc.vector.tensor_copy(bmu_f[:], bmu_ps[:])
            bls_ps = ps.tile([P, P], f32, space="PSUM", tag="mmout")
            nc.tensor.matmul(bls_ps[:], lhsT=one_row[:], rhs=vecs_t[0:1, 5*P:6*P], start=True, stop=True)
            bls_f = st.tile([P, P], f32)
            nc.vector.tensor_copy(bls_f[:], bls_ps[:])

            for wi in range(WINS):
                x2w = wk.tile([P, P], f32, tag="x1w")
                nc.vector.tensor_tensor(x2w[:], out_store[:, wi, :], a2f[:], op=AOT.mult)
                nc.vector.tensor_tensor(x2w[:], x2w[:], c2f[:], op=AOT.add)
                nc.vector.tensor_scalar(x2w[:], x2w[:], 0.0, None, op0=AOT.max)
                mups, lsps = transform(x2w[:], [Wmu_t[:], Wls_t[:]])
                mu_sb = wk.tile([P, P], f32, tag="musb")
                nc.vector.tensor_tensor(mu_sb[:], mups[:], bmu_f[:], op=AOT.add)
                nc.sync.dma_start(mu_d[wi * P : (wi + 1) * P, :], mu_sb[:])
                ls_sb = wk.tile([P, P], f32, tag="musb")
                nc.vector.tensor_tensor(ls_sb[:], lsps[:], bls_f[:], op=AOT.add)
                nc.sync.dma_start(ls_d[wi * P : (wi + 1) * P, :], ls_sb[:])
          except StopStage:
            pass

    nc.compile()
    return nc


def make_pjrt_runner(nc, in_maps):
    """Mirror bass2jax.run_bass_via_pjrt but return a reusable timed runner."""
    import jax
    import numpy as np
    from jax.sharding import Mesh, PartitionSpec, NamedSharding
    from jax.experimental.shard_map import shard_map
    from concourse import bass2jax, mybir
    from concourse.bass2jax import _bass_exec_p, install_neuronx_cc_hook

    install_neuronx_cc_hook()
    n_cores = len(in_maps)
    partition_name = nc.partition_id_tensor.name if nc.partition_id_tensor else None
    in_names, out_names, out_avals, zero_outs = [], [], [], []
    for alloc in nc.m.functions[0].allocations:
        if not isinstance(alloc, mybir.MemoryLocationSet):
            continue
        name = alloc.memorylocations[0].name
        if alloc.kind == "ExternalInput":
            if name != partition_name:
                in_names.append(name)
        elif alloc.kind == "ExternalOutput":
            shape = tuple(alloc.tensor_shape)
            dt = mybir.dt.np(alloc.dtype)
            out_avals.append(jax.core.ShapedArray(shape, dt))
            out_names.append(name)
            zero_outs.append(np.zeros(shape, dt))
    n_params = len(in_names)
    n_outs = len(out_avals)
    in_names.extend(out_names)
    if partition_name is not None:
        in_names.append(partition_name)
    donate = tuple(range(n_params, n_params + n_outs))

    def _body(*args):
        operands = list(args)
        if partition_name is not None:
            operands.append(bass2jax.partition_id_tensor())
        outs = _bass_exec_p.bind(
            *operands,
            out_avals=tuple(out_avals), in_names=tuple(in_names),
            out_names=tuple(out_names), lowering_input_output_aliases=(),
            sim_require_finite=True, sim_require_nnan=True, nc=nc)
        return tuple(outs)

    devices = jax.devices()[:n_cores]
    mesh = Mesh(np.asarray(devices), ("core",))
    in_specs = (PartitionSpec("core"),) * (n_params + n_outs)
    out_specs = (PartitionSpec("core"),) * len(out_names)
    sharded = jax.jit(
        shard_map(_body, mesh=mesh, in_specs=in_specs, out_specs=out_specs,
                  check_rep=False),
        keep_unused=True)
    sh = NamedSharding(mesh, PartitionSpec("core"))
    per_core = [[np.asarray(m[name]) for name in in_names[:n_params]]
                for m in in_maps]
    concat_in = [
        jax.device_put(
            np.concatenate([per_core[c][i] for c in range(n_cores)], axis=0), sh)
        for i in range(n_params)
    ]

    zeros_dev = [jax.device_put(
                     np.zeros((n_cores * z.shape[0], *z.shape[1:]), z.dtype), sh)
                 for z in zero_outs]

    def execute():
        return sharded(*concat_in, *zeros_dev)

    def unpack(out_arrs):
        return [
            {name: np.asarray(out_arrs[i]).reshape(n_cores, *out_avals[i].shape)[c]
             for i, name in enumerate(out_names)}
            for c in range(n_cores)
        ]
    return execute, unpack


def run_timed(inputs, iters=8, stage=5):
    """Build+compile once; run warmup + timed iterations. Returns outputs and ns/iter."""
    import time, jax
    data, CA, CB = preprocess(inputs["edge_index"], inputs["edge_weight"])
    in_maps = build_in_maps(inputs, data, CA, CB)
    hfull = in_maps[0]["hfull"]
    for c in range(NC):
        in_maps[c]["hown"] = np.ascontiguousarray(hfull[c * SLOTS : (c + 1) * SLOTS])
    nc = build_kernel(CA, CB, debug=False, stage=stage)
    execute, unpack = make_pjrt_runner(nc, in_maps)
    t0 = time.time()
    out = execute()
    jax.block_until_ready(out)
    t_first = time.time() - t0
    # timed: enqueue iters executions, block at end
    t0 = time.time()
    last = None
    for _ in range(iters):
        last = execute()
    jax.block_until_ready(last)
    t_total = time.time() - t0
    per_iter_ns = t_total / iters * 1e9
    results = unpack(last)
    mu = np.zeros((N, P), np.float32)
    ls = np.zeros((N, P), np.float32)
    for c in range(NC):
        if "mu_out" in results[c]:
            mu[c * NPC : (c + 1) * NPC] = results[c]["mu_out"][:NPC]
            ls[c * NPC : (c + 1) * NPC] = results[c]["ls_out"][:NPC]
    return (mu, ls), per_iter_ns, t_first


def run(inputs, debug=False, trace=False, stage=5):
    import time
    from concourse.bass_utils import run_bass_kernel_spmd

    t0 = time.time()
    data, CA, CB = preprocess(inputs["edge_index"], inputs["edge_weight"])
    in_maps = build_in_maps(inputs, data, CA, CB)
    h = np.asarray(inputs["h"], np.float32)
    hfull = in_maps[0]["hfull"]
    for c in range(NC):
        in_maps[c]["hown"] = np.ascontiguousarray(
            hfull[c * SLOTS : (c + 1) * SLOTS])
    prep_s = time.time() - t0

    t0 = time.time()
    nc = build_kernel(CA, CB, debug=debug, stage=stage)
    build_s = time.time() - t0

    t0 = time.time()
    res = run_bass_kernel_spmd(nc, in_maps, core_ids=list(range(NC)), trace=trace)
    run_s = time.time() - t0
    print(f"[gcn] prep {prep_s:.1f}s build {build_s:.1f}s compile+run {run_s:.1f}s",
          flush=True)

    mu = np.zeros((N, P), np.float32)
    ls = np.zeros((N, P), np.float32)
    for c in range(NC):
        if "mu_out" in res.results[c]:
            mu[c * NPC : (c + 1) * NPC] = res.results[c]["mu_out"][:NPC]
            ls[c * NPC : (c + 1) * NPC] = res.results[c]["ls_out"][:NPC]
    return (mu, ls), res


_CACHE = {}


def kernel(**inputs):
    """Full inputs -> full (mu, log_std), computed on 8 trn2 NeuronCores."""
    from concourse.bass_utils import run_bass_kernel_spmd

    data, CA, CB = preprocess(inputs["edge_index"], inputs["edge_weight"])
    in_maps = build_in_maps(inputs, data, CA, CB)
    hfull = in_maps[0]["hfull"]
    for c in range(NC):
        in_maps[c]["hown"] = np.ascontiguousarray(hfull[c * SLOTS : (c + 1) * SLOTS])

    key = (CA, CB)
    if key not in _CACHE:
        _CACHE[key] = build_kernel(CA, CB)
    nc = _CACHE[key]
    res = run_bass_kernel_spmd(nc, in_maps, core_ids=list(range(NC)))

    mu = np.zeros((N, P), np.float32)
    ls = np.zeros((N, P), np.float32)
    for c in range(NC):
        mu[c * NPC : (c + 1) * NPC] = res.results[c]["mu_out"][:NPC]
        ls[c * NPC : (c + 1) * NPC] = res.results[c]["ls_out"][:NPC]
    return (mu, ls)


# revision 4
# speedup vs baseline: 1.4945x; 1.4945x over previous
"""Trainium2 Bass kernel: 2-layer GCN encoder (BN -> GCNConv -> BN -> ReLU
-> GCNConv -> BN -> ReLU -> linear mu / log_std heads) on 8 NeuronCores.

Self-contained: kernel(**inputs) takes full inputs, shards internally
(1D node sharding, edges partitioned by destination), runs one SPMD Bass
program on cores 0-7, returns full (mu, log_std).
"""
import sys

if "/opt/trn_rl_repo" not in sys.path:
    sys.path.insert(0, "/opt/trn_rl_repo")

import numpy as np

N = 50000
NC = 8
P = 128
NPC = 6250
WINS = 49
SLOTS = WINS * P        # 6272
NTBL = NC * SLOTS       # 50176
JFULL = NTBL // P       # 392
EPS = 1e-5
A_HI = 32768
B_LO = NTBL - 32768     # 17408
G = 7                   # windows per gather batch
NB = WINS // G          # 7


def q_of(i):
    return (i // NPC) * SLOTS + (i % NPC)


def preprocess(edge_index, edge_weight):
    """Build per-core chunk structure. Returns (percore list of dicts, CA, CB)."""
    src = np.asarray(edge_index[0], np.int64)
    dst = np.asarray(edge_index[1], np.int64)
    w = np.asarray(edge_weight, np.float32)
    loop = np.arange(N, dtype=np.int64)
    src = np.concatenate([src, loop])
    dst = np.concatenate([dst, loop])
    w = np.concatenate([w, np.ones(N, np.float32)])
    qsrc = q_of(src)

    core = dst // NPC
    wloc = (dst % NPC) // P
    dloc = (dst % NPC) % P

    per = []
    for c in range(NC):
        m = core == c
        qs_c, dl_c, w_c, wl_c = qsrc[m], dloc[m], w[m], wloc[m]
        o = np.lexsort((qs_c, wl_c))
        qs_c, dl_c, w_c, wl_c = qs_c[o], dl_c[o], w_c[o], wl_c[o]
        wins = []
        start = np.searchsorted(wl_c, np.arange(WINS))
        end = np.searchsorted(wl_c, np.arange(WINS) + 1)
        for wi in range(WINS):
            qs = qs_c[start[wi]:end[wi]]
            dl = dl_c[start[wi]:end[wi]]
            ww = w_c[start[wi]:end[wi]]
            chunks = []
            i, n = 0, len(qs)
            while i < n:
                j = min(i + P, n)
                cqs = qs[i:j]
                if cqs[-1] < A_HI:
                    tag = "A"
                elif cqs[0] >= B_LO:
                    tag = "B"
                else:
                    j = i + int(np.searchsorted(cqs, A_HI))
                    assert j > i
                    cqs = qs[i:j]
                    tag = "A"
                chunks.append((tag, cqs, dl[i:j], ww[i:j]))
                i = j
            wins.append(chunks)
        per.append(wins)

    CA = max(sum(1 for t, *_ in win if t == "A") for wins in per for win in wins)
    CB = max(sum(1 for t, *_ in win if t == "B") for wins in per for win in wins)

    data = []
    for c in range(NC):
        idxA = np.zeros((WINS, CA, P), np.int32)
        dlA = np.zeros((WINS, CA, P), np.float32)
        wA = np.zeros((WINS, CA, P), np.float32)
        idxB = np.zeros((WINS, CB, P), np.int32)
        dlB = np.zeros((WINS, CB, P), np.float32)
        wB = np.zeros((WINS, CB, P), np.float32)
        for wi in range(WINS):
            ka = kb = 0
            for tag, cqs, cdl, cw in per[c][wi]:
                n = len(cqs)
                if tag == "A":
                    idxA[wi, ka, :n] = cqs
                    dlA[wi, ka, :n] = cdl
                    wA[wi, ka, :n] = cw
                    ka += 1
                else:
                    idxB[wi, kb, :n] = cqs - B_LO
                    dlB[wi, kb, :n] = cdl
                    wB[wi, kb, :n] = cw
                    kb += 1
        data.append(dict(idxA=idxA, dlA=dlA, wA=wA, idxB=idxB, dlB=dlB, wB=wB))
    return data, CA, CB


def wrap_idx16(idx_flat):
    """Edge-slot-ordered flat indices -> dma_gather int16 layout [128, n/16]."""
    n = len(idx_flat)
    assert n % 16 == 0
    base = idx_flat.reshape(n // 16, 16).T.astype(np.int16)   # [16, n/16]
    return np.tile(base, (8, 1))                               # [128, n/16]


def build_in_maps(inputs, data, CA, CB):
    h = np.asarray(inputs["h"], np.float32)
    hfull = np.zeros((NTBL, 5), np.float32)
    for c in range(NC):
        hfull[c * SLOTS : c * SLOTS + NPC] = h[c * NPC : (c + 1) * NPC]

    vecs = np.zeros((1, 8 * P), np.float32)
    vecs[0, 0*P:1*P] = np.asarray(inputs["g1"], np.float32)
    vecs[0, 1*P:2*P] = np.asarray(inputs["be1"], np.float32)
    vecs[0, 2*P:3*P] = np.asarray(inputs["g2"], np.float32)
    vecs[0, 3*P:4*P] = np.asarray(inputs["be2"], np.float32)
    vecs[0, 4*P:5*P] = np.asarray(inputs["bmu"], np.float32)
    vecs[0, 5*P:6*P] = np.asarray(inputs["bls"], np.float32)
    vecs[0, 6*P:6*P+5] = np.asarray(inputs["g0"], np.float32)
    vecs[0, 7*P:7*P+5] = np.asarray(inputs["be0"], np.float32)

    W1 = np.asarray(inputs["W1"], np.float32)              # [5,128]
    W2 = np.asarray(inputs["W2"], np.float32)              # [128,128]
    W2bf = W2.astype(np.dtype("bfloat16") if False else np.float32)
    Wmu = np.asarray(inputs["Wmu"], np.float32)
    Wls = np.asarray(inputs["Wls"], np.float32)

    in_maps = []
    for c in range(NC):
        d = data[c]
        # meta layout: [dlA | wA | dlB | wB] columns, window-major chunk order.
        # transposed to [128 rows(edges of chunk), cols(chunks)]
        def t(x, CC):
            return x.reshape(WINS * CC, P).T.copy()   # [128, WINS*CC]
        meta = np.concatenate(
            [t(d["dlA"], CA), t(d["wA"], CA), t(d["dlB"], CB), t(d["wB"], CB)],
            axis=1,
        ).astype(np.float32)
        idxA16 = wrap_idx16(d["idxA"].ravel())
        idxB16 = wrap_idx16(d["idxB"].ravel())
        in_maps.append({
            "hfull": hfull,
            "meta": meta,
            "idxA": idxA16,
            "idxB": idxB16,
            "W1": W1,
            "W2": W2,
            "Wmu": Wmu,
            "Wls": Wls,
            "vecs": vecs,
        })
    return in_maps


def build_kernel(CA, CB, debug=False, stage=5):
    import concourse.bass as bass
    import concourse.bacc as bacc
    import concourse.tile as tile
    from concourse import mybir
    from concourse.masks import make_identity

    f32 = mybir.dt.float32
    bf16 = mybir.dt.bfloat16
    i16 = mybir.dt.int16
    AOT = mybir.AluOpType

    nc = bacc.Bacc("TRN2", num_devices=NC)

    # ---- I/O ----
    hfull_d = nc.dram_tensor("hfull", [NTBL, 5], f32, kind="ExternalInput")
    meta_d = nc.dram_tensor("meta", [P, 2 * WINS * (CA + CB)], f32, kind="ExternalInput")
    idxA_d = nc.dram_tensor("idxA", [P, WINS * CA * 8], i16, kind="ExternalInput")
    idxB_d = nc.dram_tensor("idxB", [P, WINS * CB * 8], i16, kind="ExternalInput")
    W1_d = nc.dram_tensor("W1", [5, P], f32, kind="ExternalInput")
    W2_d = nc.dram_tensor("W2", [P, P], f32, kind="ExternalInput")
    Wmu_d = nc.dram_tensor("Wmu", [P, P], f32, kind="ExternalInput")
    Wls_d = nc.dram_tensor("Wls", [P, P], f32, kind="ExternalInput")
    vecs_d = nc.dram_tensor("vecs", [1, 8 * P], f32, kind="ExternalInput")
    mu_d = nc.dram_tensor("mu_out", [SLOTS, P], f32, kind="ExternalOutput")
    ls_d = nc.dram_tensor("ls_out", [SLOTS, P], f32, kind="ExternalOutput")
    if debug:
        dbg_dis_d = nc.dram_tensor("dbg_dis", [P, WINS], f32, kind="ExternalOutput")
        dbg_out1_d = nc.dram_tensor("dbg_out1", [SLOTS, P], f32, kind="ExternalOutput")
        dbg_x1_d = nc.dram_tensor("dbg_x1", [SLOTS, P], f32, kind="ExternalOutput")

    # meta column offsets
    oDlA, oWA = 0, WINS * CA
    oDlB, oWB = 2 * WINS * CA, 2 * WINS * CA + WINS * CB

    class StopStage(Exception):
        pass

    with tile.TileContext(nc) as tc:
        with (
            tc.tile_pool(name="const", bufs=1) as cp,
            tc.tile_pool(name="store", bufs=1) as st,
            tc.tile_pool(name="work", bufs=3) as wk,
            tc.tile_pool(name="spool", bufs=3) as sp,
            tc.tile_pool(name="sbig", bufs=2) as sb2,
            tc.tile_pool(name="psum", bufs=2, space="PSUM") as ps,
            tc.tile_pool(name="dram", bufs=1, space="DRAM") as dr,
        ):
          try:
            # ---------- constants ----------
            iota_f = cp.tile([P, P], f32)
            nc.gpsimd.iota(iota_f[:], pattern=[[1, P]], base=0,
                           channel_multiplier=0, allow_small_or_imprecise_dtypes=True)
            CMX = max(CA, CB)
            iota_big = cp.tile([P, CMX, P], f32)
            nc.gpsimd.iota(iota_big[:], pattern=[[0, CMX], [1, P]], base=0,
                           channel_multiplier=0, allow_small_or_imprecise_dtypes=True)

            def s_batch(oDl, oW, wi, CC, tag):
                """Batched one-hot: S[:, k, :] = (iota==dl_k) * w_k for CC chunks."""
                dlb = meta_t[:, oDl + wi * CC : oDl + (wi + 1) * CC].rearrange(
                    "p (k o) -> p k o", o=1).broadcast_to([P, CC, P])
                wb = meta_t[:, oW + wi * CC : oW + (wi + 1) * CC].rearrange(
                    "p (k o) -> p k o", o=1).broadcast_to([P, CC, P])
                tmp = sp.tile([P, CMX, P], f32, tag="stmp")
                nc.vector.tensor_tensor(tmp[:, :CC, :], iota_big[:, :CC, :], dlb,
                                        op=AOT.is_equal)
                sb_t = sp.tile([P, CC, P], bf16, tag=tag)
                nc.vector.tensor_tensor(sb_t[:], tmp[:, :CC, :], wb, op=AOT.mult)
                return sb_t
            ident = cp.tile([P, P], f32)
            make_identity(nc, ident[:])
            ones_bf = cp.tile([P, 1], bf16)
            nc.gpsimd.memset(ones_bf[:], 1.0)
            ones_f = cp.tile([P, 1], f32)
            nc.gpsimd.memset(ones_f[:], 1.0)
            one_row = cp.tile([1, P], f32)
            nc.gpsimd.memset(one_row[:], 1.0)

            meta_t = cp.tile([P, 2 * WINS * (CA + CB)], f32)
            nc.sync.dma_start(meta_t[:], meta_d[:])
            idxA_t = cp.tile([P, WINS * CA * 8], i16)
            nc.sync.dma_start(idxA_t[:], idxA_d[:])
            idxB_t = cp.tile([P, WINS * CB * 8], i16)
            nc.sync.dma_start(idxB_t[:], idxB_d[:])
            W1_t = cp.tile([5, P], f32)
            nc.sync.dma_start(W1_t[:], W1_d[:])
            W2_t = cp.tile([P, P], f32)
            nc.sync.dma_start(W2_t[:], W2_d[:])
            Wmu_t = cp.tile([P, P], f32)
            nc.sync.dma_start(Wmu_t[:], Wmu_d[:])
            Wls_t = cp.tile([P, P], f32)
            nc.sync.dma_start(Wls_t[:], Wls_d[:])
            vecs_t = cp.tile([1, 8 * P], f32)
            nc.sync.dma_start(vecs_t[:], vecs_d[:])

            # absorb DMA waits on DVE (TensorScalarPtr allows only 1 wait)
            touch = cp.tile([P, 1], f32)
            nc.vector.tensor_copy(touch[:], meta_t[:, 0:1])

            # ---------- BN0 stats from full h (replicated) ----------
            hfull_t = cp.tile([P, JFULL, 5], f32)
            nc.sync.dma_start(hfull_t[:], hfull_d[:].rearrange("(j p) d -> p j d", p=P))
            hsq = wk.tile([P, JFULL * 5], f32, tag="hsq")
            nc.scalar.square(hsq[:], hfull_t[:].rearrange("p j d -> p (j d)"))
            part_s = wk.tile([P, 5], f32, tag="part")
            nc.vector.tensor_reduce(
                part_s[:], hfull_t[:].rearrange("p j d -> p d j"),
                axis=mybir.AxisListType.X, op=AOT.add)
            part_q = wk.tile([P, 5], f32, tag="part")
            nc.vector.tensor_reduce(
                part_q[:], hsq[:].rearrange("p (j d) -> p d j", d=5),
                axis=mybir.AxisListType.X, op=AOT.add)
            s0_ps = ps.tile([1, 5], f32, space="PSUM", tag="sps")
            nc.tensor.matmul(s0_ps[:], lhsT=ones_f[:], rhs=part_s[:], start=True, stop=True)
            q0_ps = ps.tile([1, 5], f32, space="PSUM", tag="sps")
            nc.tensor.matmul(q0_ps[:], lhsT=ones_f[:], rhs=part_q[:], start=True, stop=True)

            # a0 = g0 * rsqrt(v0+eps), c0 = be0 - m0*a0   on [1,5]
            m0 = cp.tile([1, 5], f32)
            nc.vector.tensor_scalar(m0[:], s0_ps[:], 1.0 / N, None, op0=AOT.mult)
            v0 = cp.tile([1, 5], f32)
            nc.vector.tensor_scalar(v0[:], q0_ps[:], 1.0 / N, None, op0=AOT.mult)
            m0sq = wk.tile([1, 5], f32, tag="t5")
            nc.vector.tensor_tensor(m0sq[:], m0[:], m0[:], op=AOT.mult)
            nc.vector.tensor_tensor(v0[:], v0[:], m0sq[:], op=AOT.subtract)
            nc.vector.tensor_scalar(v0[:], v0[:], EPS, None, op0=AOT.add)
            rc0 = wk.tile([1, 5], f32, tag="t5")
            nc.vector.reciprocal(rc0[:], v0[:])
            rs0 = wk.tile([1, 5], f32, tag="t5")
            nc.scalar.sqrt(rs0[:], rc0[:])
            a0 = cp.tile([1, 5], f32)
            nc.vector.tensor_tensor(a0[:], rs0[:], vecs_t[0:1, 6*P:6*P+5], op=AOT.mult)
            c0 = cp.tile([1, 5], f32)
            nc.vector.tensor_tensor(c0[:], m0[:], a0[:], op=AOT.mult)
            nc.vector.tensor_tensor(c0[:], vecs_t[0:1, 7*P:7*P+5], c0[:], op=AOT.subtract)

            # broadcast a0,c0 to [128, 5] via outer product with ones
            a0f_ps = ps.tile([P, 5], f32, space="PSUM", tag="tpsmm")
            nc.tensor.matmul(a0f_ps[:], lhsT=one_row[:], rhs=a0[:], start=True, stop=True)
            a0_full = cp.tile([P, 5], f32)
            nc.vector.tensor_copy(a0_full[:], a0f_ps[:])
            c0f_ps = ps.tile([P, 5], f32, space="PSUM", tag="tpsmm")
            nc.tensor.matmul(c0f_ps[:], lhsT=one_row[:], rhs=c0[:], start=True, stop=True)
            c0_full = cp.tile([P, 5], f32)
            nc.vector.tensor_copy(c0_full[:], c0f_ps[:])

            # ---------- own-slice x0 ----------
            # hfull rows [rank*SLOTS, (rank+1)*SLOTS) -- but rank differs per core!
            # We avoid rank-dependence: each core's OWN slice in hfull is
            # provided via a separate per-core input tensor "hown".
            # (declared below, appended to I/O)

            # ---------- deg pass ----------
            deg_t = cp.tile([P, WINS], f32)
            for wi in range(WINS):
                dps = ps.tile([P, 1], f32, space="PSUM", tag="sps")
                sA = s_batch(oDlA, oWA, wi, CA, "sa")
                sB = s_batch(oDlB, oWB, wi, CB, "sb")
                nci = 0
                for k in range(CA):
                    nc.tensor.matmul(dps[:], lhsT=sA[:, k, :], rhs=ones_bf[:],
                                     start=(nci == 0), stop=False)
                    nci += 1
                for k in range(CB):
                    nci += 1
                    nc.tensor.matmul(dps[:], lhsT=sB[:, k, :], rhs=ones_bf[:],
                                     start=False, stop=(nci == CA + CB))
                nc.vector.tensor_copy(deg_t[:, wi : wi + 1], dps[:])

            # dis = (deg>0) * sqrt(1/max(deg,1e-12))   [128, WINS]
            degm = wk.tile([P, WINS], f32, tag="degm")
            nc.vector.tensor_scalar(degm[:], deg_t[:], 1e-12, None, op0=AOT.max)
            rec = wk.tile([P, WINS], f32, tag="degm")
            nc.vector.reciprocal(rec[:], degm[:])
            dsq = wk.tile([P, WINS], f32, tag="degm")
            nc.scalar.sqrt(dsq[:], rec[:])
            mask = wk.tile([P, WINS], f32, tag="degm")
            nc.vector.tensor_scalar(mask[:], deg_t[:], 0.0, None, op0=AOT.is_gt)
            dis_t = cp.tile([P, WINS], f32)
            nc.vector.tensor_tensor(dis_t[:], dsq[:], mask[:], op=AOT.mult)
            if debug:
                nc.sync.dma_start(dbg_dis_d[:], dis_t[:])

            if stage < 2:
                raise StopStage
            # ---------- helper: transform window (transpose + matmul) ----------
            def transform(src_sb, rhs_list):
                """src_sb [128, K] fp32 -> PE transpose -> [K,128] -> matmuls.
                Returns list of PSUM tiles [128, 128]."""
                kdim = src_sb.shape[-1]
                tps = ps.tile([P, P], f32, space="PSUM", tag="tpsmm")
                nc.tensor.transpose(tps[:kdim, :], src_sb, ident[:])
                tsb = wk.tile([P, P], f32, tag="tsb")
                nc.vector.tensor_copy(tsb[:kdim, :], tps[:kdim, :])
                outs = []
                for rhs in rhs_list:
                    mps = ps.tile([P, P], f32, space="PSUM", tag="tpsmm")
                    nc.tensor.matmul(mps[:], lhsT=tsb[:kdim, :], rhs=rhs,
                                     start=True, stop=True)
                    outs.append(mps)
                return outs

            # ---------- z table (layer-1, transform-first) ----------
            hown_d = nc.dram_tensor("hown", [SLOTS, 5], f32, kind="ExternalInput")
            hown_t = cp.tile([P, WINS, 5], f32)
            nc.sync.dma_start(hown_t[:], hown_d[:].rearrange("(w p) d -> p w d", p=P))
            x0_t = cp.tile([P, WINS, 5], f32)
            for dd in range(5):
                nc.vector.tensor_scalar(
                    x0_t[:, :, dd], hown_t[:, :, dd],
                    a0_full[:, dd : dd + 1], c0_full[:, dd : dd + 1],
                    op0=AOT.mult, op1=AOT.add)

            ag_in1 = dr.tile([SLOTS, P], bf16)
            tbl1 = dr.tile([NTBL, P], bf16, addr_space="Shared")
            for wi in range(WINS):
                (zps,) = transform(x0_t[:, wi, :], [W1_t[:]])
                zbf = wk.tile([P, P], bf16, tag="zbf")
                nc.vector.tensor_scalar(
                    zbf[:], zps[:], dis_t[:, wi : wi + 1], None, op0=AOT.mult)
                nc.sync.dma_start(ag_in1[wi * P : (wi + 1) * P, :], zbf[:])
            nc.gpsimd.collective_compute(
                "AllGather", AOT.bypass, replica_groups=[list(range(NC))],
                ins=[ag_in1[:]], outs=[tbl1[:]])

            if stage == 2.5:
                # probe: single gather from AG output, dump
                dbgA_d = nc.dram_tensor("dbgA", [P, G * CA, P], f32, kind="ExternalOutput")
                bufA = sb2.tile([P, G * CA, P], bf16, tag="bufA")
                nc.gpsimd.dma_gather(
                    bufA[:], tbl1[:], idxA_t[:, 0 : G * CA * 8],
                    G * CA * P, G * CA * P, P, single_packet=False)
                nc.gpsimd.dma_start(dbgA_d[:], bufA[:])
                dbgB_d = nc.dram_tensor("dbgB", [P, G * CB, P], f32, kind="ExternalOutput")
                bufB = sb2.tile([P, G * CB, P], bf16, tag="bufB")
                nc.gpsimd.dma_gather(
                    bufB[:], tbl1[B_LO:, :], idxB_t[:, 0 : G * CB * 8],
                    G * CB * P, G * CB * P, P, single_packet=False)
                nc.gpsimd.dma_start(dbgB_d[:], bufB[:])
            if stage < 3:
                raise StopStage
            # ---------- aggregation pass ----------
            out_store = st.tile([P, WINS, P], f32)

            def agg_pass(tbl):
                sum_acc = wk.tile([1, P], f32, tag="sacc")
                sq_acc = wk.tile([1, P], f32, tag="qacc")
                nc.gpsimd.memset(sum_acc[:], 0.0)
                nc.gpsimd.memset(sq_acc[:], 0.0)
                for b in range(NB):
                    w0 = b * G
                    bufA = sb2.tile([P, G * CA, P], bf16, tag="bufA")
                    nc.gpsimd.dma_gather(
                        bufA[:], tbl[:],
                        idxA_t[:, w0 * CA * 8 : (w0 + G) * CA * 8],
                        G * CA * P, G * CA * P, P, single_packet=False)
                    bufB = sb2.tile([P, G * CB, P], bf16, tag="bufB")
                    nc.gpsimd.dma_gather(
                        bufB[:], tbl[B_LO:, :],
                        idxB_t[:, w0 * CB * 8 : (w0 + G) * CB * 8],
                        G * CB * P, G * CB * P, P, single_packet=False)
                    for wi in range(w0, w0 + G):
                        agg = ps.tile([P, P], f32, space="PSUM", tag="agg", bufs=3)
                        sA = s_batch(oDlA, oWA, wi, CA, "sa")
                        sB = s_batch(oDlB, oWB, wi, CB, "sb")
                        nci = 0
                        for k in range(CA):
                            nc.tensor.matmul(
                                agg[:], lhsT=sA[:, k, :],
                                rhs=bufA[:, (wi - w0) * CA + k, :],
                                start=(nci == 0), stop=False)
                            nci += 1
                        for k in range(CB):
                            nci += 1
                            nc.tensor.matmul(
                                agg[:], lhsT=sB[:, k, :],
                                rhs=bufB[:, (wi - w0) * CB + k, :],
                                start=False, stop=(nci == CA + CB))
                        # out = dis * agg
                        outw = out_store[:, wi, :]
                        nc.vector.tensor_scalar(
                            outw, agg[:], dis_t[:, wi : wi + 1], None, op0=AOT.mult)
                        # stats
                        sq = wk.tile([P, P], f32, tag="sq")
                        nc.scalar.square(sq[:], outw)
                        sps = ps.tile([1, P], f32, space="PSUM", tag="sps")
                        nc.tensor.matmul(sps[:], lhsT=ones_f[:], rhs=outw,
                                         start=True, stop=True)
                        nc.vector.tensor_tensor(sum_acc[:], sum_acc[:], sps[:], op=AOT.add)
                        qps = ps.tile([1, P], f32, space="PSUM", tag="sps")
                        nc.tensor.matmul(qps[:], lhsT=ones_f[:], rhs=sq[:],
                                         start=True, stop=True)
                        nc.vector.tensor_tensor(sq_acc[:], sq_acc[:], qps[:], op=AOT.add)
                return sum_acc, sq_acc

            def bn_reduce(sum_acc, sq_acc, g_row, be_row, name):
                """AllReduce stats; returns (a_full, c_full) [128,128] bcast tiles."""
                bn_in = dr.tile([1, 2 * P], f32, name=f"bnin_{name}")
                bn_out = dr.tile([1, 2 * P], f32, addr_space="Shared", name=f"bnout_{name}")
                pack = wk.tile([1, 2 * P], f32, tag="bnpack")
                nc.vector.tensor_copy(pack[0:1, 0:P], sum_acc[:])
                nc.vector.tensor_copy(pack[0:1, P : 2 * P], sq_acc[:])
                nc.sync.dma_start(bn_in[:], pack[:])
                nc.gpsimd.collective_compute(
                    "AllReduce", AOT.add, replica_groups=[list(range(NC))],
                    ins=[bn_in[:]], outs=[bn_out[:]])
                bn_t = wk.tile([1, 2 * P], f32, tag="bnt")
                nc.sync.dma_start(bn_t[:], bn_out[:])
                mean = wk.tile([1, P], f32, tag="bn1")
                nc.vector.tensor_scalar(mean[:], bn_t[0:1, 0:P], 1.0 / N, None, op0=AOT.mult)
                var = wk.tile([1, P], f32, tag="bn2")
                nc.vector.tensor_scalar(var[:], bn_t[0:1, P : 2 * P], 1.0 / N, None, op0=AOT.mult)
                msq = wk.tile([1, P], f32, tag="bn3")
                nc.vector.tensor_tensor(msq[:], mean[:], mean[:], op=AOT.mult)
                nc.vector.tensor_tensor(var[:], var[:], msq[:], op=AOT.subtract)
                nc.vector.tensor_scalar(var[:], var[:], EPS, None, op0=AOT.add)
                rc = wk.tile([1, P], f32, tag="bn3")
                nc.vector.reciprocal(rc[:], var[:])
                rs = wk.tile([1, P], f32, tag="bn3")
                nc.scalar.sqrt(rs[:], rc[:])
                a_row = wk.tile([1, P], f32, tag="bn4")
                nc.vector.tensor_tensor(a_row[:], rs[:], g_row, op=AOT.mult)
                c_row = wk.tile([1, P], f32, tag="bn5")
                nc.vector.tensor_tensor(c_row[:], mean[:], a_row[:], op=AOT.mult)
                nc.vector.tensor_tensor(c_row[:], be_row, c_row[:], op=AOT.subtract)
                af_ps = ps.tile([P, P], f32, space="PSUM", tag="tpsmm")
                nc.tensor.matmul(af_ps[:], lhsT=one_row[:], rhs=a_row[:], start=True, stop=True)
                a_full = st.tile([P, P], f32, name=f"afull_{name}")
                nc.vector.tensor_copy(a_full[:], af_ps[:])
                cf_ps = ps.tile([P, P], f32, space="PSUM", tag="tpsmm")
                nc.tensor.matmul(cf_ps[:], lhsT=one_row[:], rhs=c_row[:], start=True, stop=True)
                c_full = st.tile([P, P], f32, name=f"cfull_{name}")
                nc.vector.tensor_copy(c_full[:], cf_ps[:])
                return a_full, c_full

            # ----- layer 1 -----
            sum1, sq1 = agg_pass(tbl1)
            a1f, c1f = bn_reduce(sum1, sq1, vecs_t[0:1, 0:P], vecs_t[0:1, P:2*P], "bn1")
            if debug:
                nc.sync.dma_start(
                    dbg_out1_d[:].rearrange("(w p) d -> p w d", p=P), out_store[:])

            if stage < 4:
                raise StopStage
            ag_in2 = dr.tile([SLOTS, P], bf16)
            tbl2 = dr.tile([NTBL, P], bf16, addr_space="Shared")
            for wi in range(WINS):
                x1w = wk.tile([P, P], f32, tag="x1w")
                nc.vector.tensor_tensor(x1w[:], out_store[:, wi, :], a1f[:], op=AOT.mult)
                nc.vector.tensor_tensor(x1w[:], x1w[:], c1f[:], op=AOT.add)
                nc.vector.tensor_scalar(x1w[:], x1w[:], 0.0, None, op0=AOT.max)
                if debug:
                    nc.sync.dma_start(dbg_x1_d[wi * P : (wi + 1) * P, :], x1w[:])
                (w2ps,) = transform(x1w[:], [W2_t[:]])
                tbf = wk.tile([P, P], bf16, tag="zbf")
                nc.vector.tensor_scalar(
                    tbf[:], w2ps[:], dis_t[:, wi : wi + 1], None, op0=AOT.mult)
                nc.sync.dma_start(ag_in2[wi * P : (wi + 1) * P, :], tbf[:])
            nc.gpsimd.collective_compute(
                "AllGather", AOT.bypass, replica_groups=[list(range(NC))],
                ins=[ag_in2[:]], outs=[tbl2[:]])

            # ----- layer 2 -----
            sum2, sq2 = agg_pass(tbl2)
            a2f, c2f = bn_reduce(sum2, sq2, vecs_t[0:1, 2*P:3*P], vecs_t[0:1, 3*P:4*P], "bn2")

            if stage < 5:
                raise StopStage
            # bias rows for heads -> bcast tiles
            bmu_ps = ps.tile([P, P], f32, space="PSUM", tag="tpsmm")
            nc.tensor.matmul(bmu_ps[:], lhsT=one_row[:], rhs=vecs_t[0:1, 4*P:5*P], start=True, stop=True)
            bmu_f = st.tile([P, P], f32)
            nc.vector.tensor_copy(bmu_f[:], bmu_ps[:])
            bls_ps = ps.tile([P, P], f32, space="PSUM", tag="tpsmm")
            nc.tensor.matmul(bls_ps[:], lhsT=one_row[:], rhs=vecs_t[0:1, 5*P:6*P], start=True, stop=True)
            bls_f = st.tile([P, P], f32)
            nc.vector.tensor_copy(bls_f[:], bls_ps[:])

            for wi in range(WINS):
                x2w = wk.tile([P, P], f32, tag="x1w")
                nc.vector.tensor_tensor(x2w[:], out_store[:, wi, :], a2f[:], op=AOT.mult)
                nc.vector.tensor_tensor(x2w[:], x2w[:], c2f[:], op=AOT.add)
                nc.vector.tensor_scalar(x2w[:], x2w[:], 0.0, None, op0=AOT.max)
                mups, lsps = transform(x2w[:], [Wmu_t[:], Wls_t[:]])
                mu_sb = wk.tile([P, P], f32, tag="musb")
                nc.vector.tensor_tensor(mu_sb[:], mups[:], bmu_f[:], op=AOT.add)
                nc.sync.dma_start(mu_d[wi * P : (wi + 1) * P, :], mu_sb[:])
                ls_sb = wk.tile([P, P], f32, tag="musb")
                nc.vector.tensor_tensor(ls_sb[:], lsps[:], bls_f[:], op=AOT.add)
                nc.sync.dma_start(ls_d[wi * P : (wi + 1) * P, :], ls_sb[:])
          except StopStage:
            pass

    nc.compile()
    return nc


def make_pjrt_runner(nc, in_maps):
    """Mirror bass2jax.run_bass_via_pjrt but return a reusable timed runner."""
    import jax
    import numpy as np
    from jax.sharding import Mesh, PartitionSpec, NamedSharding
    from jax.experimental.shard_map import shard_map
    from concourse import bass2jax, mybir
    from concourse.bass2jax import _bass_exec_p, install_neuronx_cc_hook

    install_neuronx_cc_hook()
    n_cores = len(in_maps)
    partition_name = nc.partition_id_tensor.name if nc.partition_id_tensor else None
    in_names, out_names, out_avals, zero_outs = [], [], [], []
    for alloc in nc.m.functions[0].allocations:
        if not isinstance(alloc, mybir.MemoryLocationSet):
            continue
        name = alloc.memorylocations[0].name
        if alloc.kind == "ExternalInput":
            if name != partition_name:
                in_names.append(name)
        elif alloc.kind == "ExternalOutput":
            shape = tuple(alloc.tensor_shape)
            dt = mybir.dt.np(alloc.dtype)
            out_avals.append(jax.core.ShapedArray(shape, dt))
            out_names.append(name)
            zero_outs.append(np.zeros(shape, dt))
    n_params = len(in_names)
    n_outs = len(out_avals)
    in_names.extend(out_names)
    if partition_name is not None:
        in_names.append(partition_name)
    donate = tuple(range(n_params, n_params + n_outs))

    def _body(*args):
        operands = list(args)
        if partition_name is not None:
            operands.append(bass2jax.partition_id_tensor())
        outs = _bass_exec_p.bind(
            *operands,
            out_avals=tuple(out_avals), in_names=tuple(in_names),
            out_names=tuple(out_names), lowering_input_output_aliases=(),
            sim_require_finite=True, sim_require_nnan=True, nc=nc)
        return tuple(outs)

    devices = jax.devices()[:n_cores]
    mesh = Mesh(np.asarray(devices), ("core",))
    in_specs = (PartitionSpec("core"),) * (n_params + n_outs)
    out_specs = (PartitionSpec("core"),) * len(out_names)
    sharded = jax.jit(
        shard_map(_body, mesh=mesh, in_specs=in_specs, out_specs=out_specs,
                  check_rep=False),
        keep_unused=True)
    sh = NamedSharding(mesh, PartitionSpec("core"))
    per_core = [[np.asarray(m[name]) for name in in_names[:n_params]]
                for m in in_maps]
    concat_in = [
        jax.device_put(
            np.concatenate([per_core[c][i] for c in range(n_cores)], axis=0), sh)
        for i in range(n_params)
    ]

    zeros_dev = [jax.device_put(
                     np.zeros((n_cores * z.shape[0], *z.shape[1:]), z.dtype), sh)
                 for z in zero_outs]

    def execute():
        return sharded(*concat_in, *zeros_dev)

    def unpack(out_arrs):
        return [
            {name: np.asarray(out_arrs[i]).reshape(n_cores, *out_avals[i].shape)[c]
             for i, name in enumerate(out_names)}
            for c in range(n_cores)
        ]
    return execute, unpack


def run_timed(inputs, iters=8, stage=5):
    """Build+compile once; run warmup + timed iterations. Returns outputs and ns/iter."""
    import time, jax
    data, CA, CB = preprocess(inputs["edge_index"], inputs["edge_weight"])
    in_maps = build_in_maps(inputs, data, CA, CB)
    hfull = in_maps[0]["hfull"]
    for c in range(NC):
        in_maps[c]["hown"] = np.ascontiguousarray(hfull[c * SLOTS : (c + 1) * SLOTS])
    nc = build_kernel(CA, CB, debug=False, stage=stage)
    execute, unpack = make_pjrt_runner(nc, in_maps)
    t0 = time.time()
    out = execute()
    jax.block_until_ready(out)
    t_first = time.time() - t0
    # timed: enqueue iters executions, block at end
    t0 = time.time()
    last = None
    for _ in range(iters):
        last = execute()
    jax.block_until_ready(last)
    t_total = time.time() - t0
    per_iter_ns = t_total / iters * 1e9
    results = unpack(last)
    mu = np.zeros((N, P), np.float32)
    ls = np.zeros((N, P), np.float32)
    for c in range(NC):
        if "mu_out" in results[c]:
            mu[c * NPC : (c + 1) * NPC] = results[c]["mu_out"][:NPC]
            ls[c * NPC : (c + 1) * NPC] = results[c]["ls_out"][:NPC]
    return (mu, ls), per_iter_ns, t_first


def run(inputs, debug=False, trace=False, stage=5):
    import time
    from concourse.bass_utils import run_bass_kernel_spmd

    t0 = time.time()
    data, CA, CB = preprocess(inputs["edge_index"], inputs["edge_weight"])
    in_maps = build_in_maps(inputs, data, CA, CB)
    h = np.asarray(inputs["h"], np.float32)
    hfull = in_maps[0]["hfull"]
    for c in range(NC):
        in_maps[c]["hown"] = np.ascontiguousarray(
            hfull[c * SLOTS : (c + 1) * SLOTS])
    prep_s = time.time() - t0

    t0 = time.time()
    nc = build_kernel(CA, CB, debug=debug, stage=stage)
    build_s = time.time() - t0

    t0 = time.time()
    res = run_bass_kernel_spmd(nc, in_maps, core_ids=list(range(NC)), trace=trace)
    run_s = time.time() - t0
    print(f"[gcn] prep {prep_s:.1f}s build {build_s:.1f}s compile+run {run_s:.1f}s",
          flush=True)

    mu = np.zeros((N, P), np.float32)
    ls = np.zeros((N, P), np.float32)
    for c in range(NC):
        if "mu_out" in res.results[c]:
            mu[c * NPC : (c + 1) * NPC] = res.results[c]["mu_out"][:NPC]
            ls[c * NPC : (c + 1) * NPC] = res.results[c]["ls_out"][:NPC]
    return (mu, ls), res


_CACHE = {}


def kernel(**inputs):
    """Full inputs -> full (mu, log_std), computed on 8 trn2 NeuronCores."""
    from concourse.bass_utils import run_bass_kernel_spmd

    data, CA, CB = preprocess(inputs["edge_index"], inputs["edge_weight"])
    in_maps = build_in_maps(inputs, data, CA, CB)
    hfull = in_maps[0]["hfull"]
    for c in range(NC):
        in_maps[c]["hown"] = np.ascontiguousarray(hfull[c * SLOTS : (c + 1) * SLOTS])

    key = (CA, CB)
    if key not in _CACHE:
        _CACHE[key] = build_kernel(CA, CB)
    nc = _CACHE[key]
    res = run_bass_kernel_spmd(nc, in_maps, core_ids=list(range(NC)))

    mu = np.zeros((N, P), np.float32)
    ls = np.zeros((N, P), np.float32)
    for c in range(NC):
        mu[c * NPC : (c + 1) * NPC] = res.results[c]["mu_out"][:NPC]
        ls[c * NPC : (c + 1) * NPC] = res.results[c]["ls_out"][:NPC]
    return (mu, ls)
